# revision 1
# baseline (speedup 1.0000x reference)
"""Causal self-attention (QK-RMSNorm + rotary + value-embed blend) on 8 trn2 cores.

Sharding: 8 cores = 4 batches x 2 head-halves (8 heads each).
Host pre-transposes/casts inputs to fp16; device computes a per-core output
partial [1024, 1024] fp32 (output projection contracted over its 512 hdim
columns); host sums the two partials per batch.

Per-core kernel structure (pipelined per head-pair g in 0..3):
  proj(g): q|k|v = x @ W[:, pair-cols]  (fp16 matmuls, one fp32 psum group)
  RMS-norm scale batched per pair (ACT square -> DVE reduce -> ln/exp rsqrt),
  applied in-place on the fp16 copy; rotary on the 32 rotating lanes per head
  transpose q,k via a DRAM roundtrip + xbar transpose DMA (fp16)
  scores^T[kpos, q] = K^T q per (head, kt): matmuls restricted to q >= kt*128;
  the diagonal 128-block gets an additive -C*max(0,k-q) ramp from a rank-128
  mask matmul (Am.T @ Bm) accumulated into the same psum, so exp kills the
  upper triangle with no elementwise mask op
  exp via ACT straight from psum (scale=0.12 fused), output fp16, packed
  causally into ET
  AV: y[q,:] accumulates ET[kt]^T @ [V|1] over kt for two q-tiles per psum
  bank; softmax denominator comes from the appended ones column; divide via
  per-partition reciprocal + free-dim-broadcast multiply
  y transposed per pair (jt == g) via DRAM roundtrip
Then out_partial = y @ woT (contract the local 512 hdim cols); host sums the
two per-batch partials.

Numerics: QK RMS-norm bounds scores (|s| <= 0.12*64), so softmax needs no
max-subtraction; rsqrt is computed as exp(-0.5*ln(x)) so ACT only ever loads
the ln/exp table set (no table thrash against the attention exp).
"""

import sys

if "/opt/trn_rl_repo" not in sys.path:
    sys.path.insert(0, "/opt/trn_rl_repo")

import numpy as np

import concourse.bass as bass
import concourse.mybir as mybir
import concourse.tile as tile
from concourse.bass import ds, ts

P = 128
B, T, D = 4, 1024, 1024
H, DH = 16, 64
H8 = 8            # heads per core
NG = H8 // 2      # head pairs
ATTN_SCALE = 0.12
EPS = 1e-6
N_CORES = 8
TT_N = T // P     # 8 t-tiles
DT_N = D // P     # 8 d-tiles (contraction)
KT_N = T // P     # 8 kpos tiles
RT_N = (H8 * DH) // P  # 4 row-tiles of Q^T/K^T
JT_N = (H8 * DH) // P  # 4 j-tiles for out-proj contraction

f16 = mybir.dt.float16
f32 = mybir.dt.float32


def split_sync_waits(nc, max_waits=1):
    """This container's walrus rejects >1 sync-wait per instruction; spill
    extra waits onto preceding NoOps on the same engine."""
    n = 0
    for fn in nc.m.functions:
        for bb in fn.blocks:
            new_insts = []
            for inst in bb.instructions:
                si = getattr(inst, "sync_info", None)
                if si is not None and si.on_wait and len(si.on_wait) > max_waits:
                    waits = list(si.on_wait)
                    extra, keep = waits[:-max_waits], waits[-max_waits:]
                    for w in extra:
                        nop = mybir.InstNoOp(
                            name=nc.get_next_instruction_name(),
                            sync_info=mybir.SyncInfo(on_wait=[w], on_update=[]),
                            bass_nofuse=True,
                            engine=inst.engine,
                        )
                        nc.register_instruction(nop)
                        new_insts.append(nop)
                        n += 1
                    si.on_wait = keep
                new_insts.append(inst)
            bb.instructions[:] = new_insts
    return n


def build_nc(reps=1):
    nc = bass.Bass()

    xT = nc.declare_dram_parameter("xT", [D, T], f16, isOutput=False)
    # per-pair contiguous fused qkv weights: [D, pair, (q|k|v)*128]
    wqkv = nc.declare_dram_parameter("wqkv", [D, NG, 384], f16, isOutput=False)
    woT = nc.declare_dram_parameter("woT", [H8 * DH, D], f16, isOutput=False)
    ve = nc.declare_dram_parameter("ve", [T, H8 * DH], f16, isOutput=False)
    cosd = nc.declare_dram_parameter("cosd", [T, 64], f16, isOutput=False)
    sind = nc.declare_dram_parameter("sind", [T, 64], f16, isOutput=False)
    # causal ramp-mask factors: (Am.T @ Bm)[k, q] = -C * max(0, k - q)
    amask = nc.declare_dram_parameter("amask", [P, P], f16, isOutput=False)
    bmask = nc.declare_dram_parameter("bmask", [P, P], f16, isOutput=False)
    out = nc.declare_dram_parameter("out", [T, D], f32, isOutput=True)

    xT_v = xT.rearrange("(dt p) t -> p dt t", p=P)
    wqkv_v = wqkv.rearrange("(dt p) g r -> p dt g r", p=P)
    wo_v = woT.rearrange("(jt p) i -> p jt i", p=P)
    ve_v = ve.rearrange("(tt p) r -> p tt r", p=P)
    cos_v = cosd.rearrange("(tt p) e -> p tt e", p=P)
    sin_v = sind.rearrange("(tt p) e -> p tt e", p=P)
    out_v = out.rearrange("(tt p) i -> p tt i", p=P)

    with tile.TileContext(nc) as tc:
        import contextlib

        with contextlib.ExitStack() as ctx:
            const = ctx.enter_context(tc.tile_pool(name="const", bufs=1))
            big = ctx.enter_context(tc.tile_pool(name="big", bufs=1))

            # -------- persistent tiles --------
            xT_sb = big.tile([P, DT_N, T], f16)
            wqkv_sb = const.tile([P, DT_N, NG, 384], f16)
            wo_sb = const.tile([P, JT_N, D], f16)
            ve_sb = const.tile([P, TT_N, H8 * DH], f16)
            cos_sb = const.tile([P, TT_N, 64], f16)
            sin_sb = const.tile([P, TT_N, 64], f16)
            am_sb = const.tile([P, P], f16)
            bm_sb = const.tile([P, P], f16)

            # loads: pair-0 weights + x first so the pipeline starts early
            nc.sync.dma_start(out=wqkv_sb[:, :, 0, :], in_=wqkv_v[:, :, 0, :])
            nc.sync.dma_start(out=xT_sb[:], in_=xT_v[:])
            nc.sync.dma_start(out=ve_sb[:], in_=ve_v[:])
            nc.sync.dma_start(out=cos_sb[:], in_=cos_v[:])
            nc.sync.dma_start(out=sin_sb[:], in_=sin_v[:])
            nc.sync.dma_start(out=am_sb[:], in_=amask[:])
            nc.sync.dma_start(out=bm_sb[:], in_=bmask[:])
            for g in range(1, NG):
                nc.sync.dma_start(
                    out=wqkv_sb[:, :, g, :], in_=wqkv_v[:, :, g, :])
            nc.sync.dma_start(out=wo_sb[:], in_=wo_v[:])

            vp = big.tile([P, KT_N, H8, DH + 1], f16)  # V blended + ones col
            y16 = big.tile([P, TT_N, H8 * DH], f16)
            QT_sb = big.tile([P, RT_N, T], f16)      # [r, t] fp16
            KT_sb = big.tile([P, RT_N, T], f16)
            yT_sb = big.tile([P, JT_N, T], f16)

            nc.vector.memset(vp[:, :, :, DH], 1.0)
            eps_sb = const.tile([P, 1], f32)
            nc.vector.memset(eps_sb[:], EPS)

            for _rep in range(reps):
                with tc.tile_pool(name="projps", bufs=3, space="PSUM") as psB, \
                     tc.tile_pool(name="stps", bufs=1, space="PSUM") as psST, \
                     tc.tile_pool(name="avps", bufs=1, space="PSUM") as psAV, \
                     tc.tile_pool(name="qk", bufs=2) as qkp, \
                     tc.tile_pool(name="etp", bufs=3) as etp, \
                     tc.tile_pool(name="qkdr", bufs=2, space="DRAM") as qkdr, \
                     tc.tile_pool(name="stat", bufs=4) as statp:
                    for g in range(NG):
                        gc = ts(g, P)  # this pair's 128 cols in q/k/v row space
                        QKg = qkp.tile([P, TT_N, 2 * P], f16, tag="qkg")
                        Qg = QKg[:, :, 0:P]
                        Kg = QKg[:, :, P : 2 * P]
                        # ---------- projection (+v blend, raw qk copy) ----------
                        sqg = statp.tile([P, TT_N, 256], f16, tag="sqg")
                        for tt in range(TT_N):
                            pj = psB.tile([P, 384], f32, tag="pqkv")
                            # single psum accumulation group for the whole
                            # bank (q,k,v ranges interleave; per-element
                            # has_written handles first-write-overwrite)
                            for dt in range(DT_N):
                                lx = xT_sb[:, dt, ts(tt, P)]
                                nc.tensor.matmul(
                                    pj[:, 0:128], lx, wqkv_sb[:, dt, g, 0:128],
                                    start=(dt == 0), stop=False)
                                nc.tensor.matmul(
                                    pj[:, 128:256], lx,
                                    wqkv_sb[:, dt, g, 128:256],
                                    start=False, stop=False)
                                nc.tensor.matmul(
                                    pj[:, 256:384], lx,
                                    wqkv_sb[:, dt, g, 256:384],
                                    start=False, stop=(dt == DT_N - 1))
                            # v blend -> vp
                            nc.vector.tensor_tensor(
                                vp[:, tt, 2 * g : 2 * g + 2, 0:DH],
                                pj[:, 256:384].rearrange("p (h e) -> p h e", h=2),
                                ve_sb[:, tt, gc].rearrange("p (h e) -> p h e", h=2),
                                mybir.AluOpType.add,
                            )
                            # raw q,k copy (normalized later, batched)
                            nc.any.tensor_copy(out=QKg[:, tt, :], in_=pj[:, 0:256])
                            nc.scalar.square(sqg[:, tt, :], pj[:, 0:256])
                        # ---------- batched RMS-norm scale ----------
                        ms = statp.tile([P, TT_N, 4], f32, tag="ms")
                        nc.vector.reduce_sum(
                            ms[:],
                            sqg.rearrange("p tt (h e) -> p tt h e", h=4),
                            axis=mybir.AxisListType.X,
                        )
                        lnv = statp.tile([P, TT_N, 4], f32, tag="lnv")
                        nc.scalar.activation(
                            lnv[:], ms[:], mybir.ActivationFunctionType.Ln,
                            bias=eps_sb[:], scale=1.0 / DH,
                        )
                        scl = statp.tile([P, TT_N, 4], f32, tag="scl")
                        nc.scalar.activation(
                            scl[:], lnv[:], mybir.ActivationFunctionType.Exp,
                            scale=-0.5,
                        )
                        nc.vector.tensor_tensor(
                            QKg.rearrange("p tt (h e) -> p tt h e", h=4),
                            QKg.rearrange("p tt (h e) -> p tt h e", h=4),
                            scl[:, :, :, None].to_broadcast((P, TT_N, 4, DH)),
                            mybir.AluOpType.mult,
                        )
                        # ---------- rotary (batched over all tt) ----------
                        for Xg in (Qg, Kg):
                            # rotating cols as [P, tt, hh=4(h,half), 16] (3 free
                            # dims - the ISA limit); cos64/sin64 tables are
                            # pre-tiled on host to the same [*, 64] layout
                            rot = Xg.rearrange(
                                "p tt (hh eh e) -> p tt hh eh e", hh=4, eh=2,
                            )[:, :, :, 0, :]  # [P, 8, 4, 16]
                            qsw = statp.tile([P, TT_N, 4, 16], f16, tag="qsw")
                            nc.vector.tensor_copy(
                                qsw[:, :, 0::2, :], rot[:, :, 1::2, :])
                            nc.vector.tensor_copy(
                                qsw[:, :, 1::2, :], rot[:, :, 0::2, :])
                            t1 = statp.tile([P, TT_N, 4, 16], f16, tag="t1")
                            nc.vector.tensor_tensor(
                                t1[:], rot,
                                cos_sb.rearrange(
                                    "p tt (hh e) -> p tt hh e", hh=4),
                                mybir.AluOpType.mult,
                            )
                            t2 = statp.tile([P, TT_N, 4, 16], f16, tag="t2")
                            nc.vector.tensor_tensor(
                                t2[:], qsw[:],
                                sin_sb.rearrange(
                                    "p tt (hh e) -> p tt hh e", hh=4),
                                mybir.AluOpType.mult,
                            )
                            nc.vector.tensor_tensor(
                                rot, t1[:], t2[:], mybir.AluOpType.add)
                        # ---------- transpose q,k via DRAM roundtrip ----------
                        qk_dr = qkdr.tile([T, 2 * P], f16, tag="qkdr")
                        nc.sync.dma_start(
                            out=qk_dr.rearrange("(tt p) r -> p tt r", p=P),
                            in_=QKg[:],
                        )
                        nc.sync.dma_start_transpose(
                            QT_sb[:, g, :], qk_dr[:, 0:P])
                        nc.sync.dma_start_transpose(
                            KT_sb[:, g, :], qk_dr[:, P : 2 * P])

                        # ---------- scores^T + exp (causal mask via ramp mm) ----
                        # ET packed causally: kt block holds q in [kt*128, T)
                        etoff = [kt * T - 64 * kt * (kt - 1) for kt in range(KT_N + 1)]
                        ET = etp.tile([P, 2, etoff[KT_N]], f16, tag="et")
                        for kt in range(KT_N):
                            qlo = kt * P
                            pst = psST.tile([P, 2, T], f32, tag="st")
                            for hb in range(2):
                                lo, hi = hb * 64, hb * 64 + 64
                                for qh in range(2):
                                    qs = max(qh * 512, qlo)
                                    qe = (qh + 1) * 512
                                    if qs >= qe:
                                        continue
                                    diag = qs == qlo
                                    nc.tensor.matmul(
                                        pst[:, hb, ds(qs, qe - qs)],
                                        KT_sb[lo:hi, g, ts(kt, P)],
                                        QT_sb[lo:hi, g, ds(qs, qe - qs)],
                                        start=True, stop=not diag,
                                    )
                                    if diag:
                                        # additive -C*max(0, k-q) ramp kills
                                        # the upper triangle under exp
                                        nc.tensor.matmul(
                                            pst[:, hb, ds(qlo, P)],
                                            am_sb[:], bm_sb[:],
                                            start=False, stop=True,
                                        )
                            nc.scalar.activation(
                                ET[:, :, ds(etoff[kt], T - qlo)],
                                pst[:, :, ds(qlo, T - qlo)],
                                mybir.ActivationFunctionType.Exp,
                                scale=ATTN_SCALE,
                            )
                        # ---------- AV + divide (two q-tiles per psum bank) ----
                        for q2 in range(TT_N // 2):
                            pav = psAV.tile([P, 260], f32, tag="av")
                            first, last = None, None
                            mms = []
                            for sub in range(2):
                                qt = 2 * q2 + sub
                                for hb in range(2):
                                    for kt in range(qt + 1):
                                        mms.append((sub, qt, hb, kt))
                            for i, (sub, qt, hb, kt) in enumerate(mms):
                                nc.tensor.matmul(
                                    pav[:, ds(sub * 130 + hb * 65, 65)],
                                    ET[:, hb, ds(etoff[kt] + (qt - kt) * P, P)],
                                    vp[:, kt, 2 * g + hb, :],
                                    start=(i == 0),
                                    stop=(i == len(mms) - 1),
                                )
                            pavv = pav.rearrange("p (s h c) -> p s h c", s=2, h=2)
                            r = statp.tile([P, 2, 2], f32, tag="r")
                            nc.vector.reciprocal(r[:], pavv[:, :, :, DH : DH + 1])
                            nc.vector.tensor_tensor(
                                y16[:, ds(2 * q2, 2), gc].rearrange(
                                    "p s (h e) -> p s h e", h=2),
                                pavv[:, :, :, 0:DH],
                                r[:, :, :, None].to_broadcast((P, 2, 2, DH)),
                                mybir.AluOpType.mult,
                            )
                        # transpose this pair's y columns (jt == g)
                        y_dr = qkdr.tile([T, P], f16, tag="ydr")
                        nc.sync.dma_start(
                            out=y_dr.rearrange("(tt p) r -> p tt r", p=P),
                            in_=y16[:, :, gc],
                        )
                        nc.sync.dma_start_transpose(yT_sb[:, g, :], y_dr[:])
                # ================= output projection =====================
                with tc.tile_pool(name="outps", bufs=2, space="PSUM") as psF, \
                     tc.tile_pool(name="outstage", bufs=2) as osp:
                    for tt2 in range(TT_N // 2):
                        osb = osp.tile([P, 2, D], f32, tag="osb")
                        for sub in range(2):
                            tt = 2 * tt2 + sub
                            for ic in range(2):
                                po = psF.tile([P, 512], f32, tag="po")
                                for jt in range(JT_N):
                                    nc.tensor.matmul(
                                        po[:],
                                        yT_sb[:, jt, ts(tt, P)],
                                        wo_sb[:, jt, ds(ic * 512, 512)],
                                        start=(jt == 0), stop=(jt == JT_N - 1),
                                    )
                                nc.any.tensor_copy(
                                    out=osb[:, sub, ds(ic * 512, 512)], in_=po[:]
                                )
                        nc.gpsimd.dma_start(
                            out=out_v[:, ds(2 * tt2, 2), :], in_=osb[:]
                        )

    split_sync_waits(nc)
    return nc


def make_core_inputs(x, qkvo_w, value_embeds, lambda_v):
    """Host-side prep: returns list of per-core input dicts (fp16)."""
    x = np.asarray(x)
    qkvo_w = np.asarray(qkvo_w)
    value_embeds = np.asarray(value_embeds)
    lambda_v = np.asarray(lambda_v)

    freq = (1.0 / 1024.0) ** np.linspace(0.0, 1.0, DH // 4, dtype=np.float32)
    theta = np.arange(T, dtype=np.float32)[:, None] * freq[None, :]  # [T, 16]
    cos = np.cos(theta).astype(np.float32)
    sin = np.sin(theta).astype(np.float32)
    # [T, 64] pre-tiled over (h,half) pairs: cos repeats, sin alternates sign
    cos64 = np.concatenate([cos, cos, cos, cos], axis=1).astype(np.float16)
    sin64 = np.concatenate([sin, -sin, sin, -sin], axis=1).astype(np.float16)
    # additive causal ramp mask: (amask.T @ bmask)[k, q] = -2000*max(0, k-q)
    jj = np.arange(P)
    amask_np = (jj[None, :] >= jj[:, None]).astype(np.float16)   # [j, k]
    bmask_np = (-2000.0 * (jj[:, None] > jj[None, :])).astype(np.float16)  # [j, q]

    in_maps = []
    for c in range(N_CORES):
        b, hh = c // 2, c % 2
        R = slice(hh * H8 * DH, (hh + 1) * H8 * DH)
        wq = qkvo_w[0][R].T  # [D, 512]
        wk = qkvo_w[1][R].T
        wv = (lambda_v[0] * qkvo_w[2][R]).T
        # [D, NG, 384]: per pair the 128 q cols, 128 k cols, 128 v cols
        wqkv = np.empty((D, NG, 384), dtype=np.float16)
        for g in range(NG):
            wqkv[:, g, 0:128] = wq[:, g * 128 : (g + 1) * 128]
            wqkv[:, g, 128:256] = wk[:, g * 128 : (g + 1) * 128]
            wqkv[:, g, 256:384] = wv[:, g * 128 : (g + 1) * 128]
        in_maps.append({
            "xT": np.ascontiguousarray(x[b].T).astype(np.float16),
            "wqkv": wqkv,
            "woT": np.ascontiguousarray(qkvo_w[3][:, R].T).astype(np.float16),
            "ve": (lambda_v[1] * value_embeds[:T, R]).astype(np.float16),
            "cosd": cos64,
            "sind": sin64,
            "amask": amask_np,
            "bmask": bmask_np,
        })
    return in_maps


_NC_CACHE = {}


def _get_nc(reps=1):
    if reps not in _NC_CACHE:
        _NC_CACHE[reps] = build_nc(reps)
    return _NC_CACHE[reps]


def kernel(x, qkvo_w, value_embeds, lambda_v):
    from concourse.bass_utils import run_bass_kernel_spmd

    nc = _get_nc()
    in_maps = make_core_inputs(x, qkvo_w, value_embeds, lambda_v)
    res = run_bass_kernel_spmd(nc, in_maps, list(range(N_CORES))).results
    out = np.empty((B, T, D), dtype=np.float32)
    for b in range(B):
        out[b] = res[2 * b]["out"] + res[2 * b + 1]["out"]
    return out



# revision 16
# speedup vs baseline: 1.0140x; 1.0140x over previous
"""Causal self-attention (QK-RMSNorm + rotary + value-embed blend) on 8 trn2 cores.

Sharding: 8 cores = 4 batches x 2 head-halves (8 heads each); host sums the
two fp16 partials per batch (out proj contracted over local 512 hdim cols).

v2 pipeline (vs v1 baseline at 128.5us):
  - QKV projection in fp8-e4m3 "bothsplit" DoubleRow matmuls: x and 64*w are
    each shipped as (hi, hi/16, lo*16) fp8 triplets; psum accumulates
    hi*hi + (hi/16)*(lo*16) + (lo*16)*(hi/16) over dt-pairs at 0.5 cyc/row
    (0.75x fp16 cycles, ~1e-3 error, measured).  The 64x weight scale folds
    out through QK RMS-norm; the value path divides by 64 in the blend.
  - Software-pipelined emission: PE stream is proj(0) scores(0) proj(1)
    av(0) scores(1) proj(2) ... so ACT exp(g) (the near-co-bottleneck) hides
    under proj(g+1), and psum stays within 8 banks
    (proj 2x1 + scores 2x2 + av 2x1).
  - scores psum in [P,2,512] q-chunks (2 banks) double-buffered; exp straight
    from psum with fused 0.12 scale; causal packing of ET unchanged.
  - SBUF->SBUF xbar transpose DMAs (no DRAM roundtrip).
  - engine rebalance: squares on DVE (fp16 2x), v-blend + psum copies +
    divide on Pool (scalar_tensor_tensor folds the 1/64), rotary as 4 DVE
    ops via shifted-slice adds, out staging copies on Pool, out stored fp16.
  - DMA spread: x t-chunks 0-3 on ACT queue, rest + weights + transposes on
    SP, ve/cos/sin + output on Pool, ordered by first use.
"""

import sys

if "/opt/trn_rl_repo" not in sys.path:
    sys.path.insert(0, "/opt/trn_rl_repo")

import numpy as np

import concourse.bass as bass
import concourse.mybir as mybir
import concourse.tile as tile
from concourse.bass import ds, ts

P = 128
B, T, D = 4, 1024, 1024
H, DH = 16, 64
H8 = 8            # heads per core
NG = H8 // 2      # head pairs
ATTN_SCALE = 0.12
EPS = 1e-6
N_CORES = 8
TT_N = T // P     # 8 t-tiles
DT_N = D // P     # 8 d-tiles (contraction)
KT_N = T // P
JT_N = (H8 * DH) // P  # 4 j-tiles for out-proj contraction
WSC = 64.0        # weight pre-scale for fp8

f8 = mybir.dt.float8e4
f16 = mybir.dt.float16
f32 = mybir.dt.float32
DR = mybir.MatmulPerfMode.DoubleRow
ETOFF = [kt * T - 64 * kt * (kt - 1) for kt in range(KT_N + 1)]


def split_sync_waits(nc, max_waits=1):
    """This container's walrus rejects >1 sync-wait per instruction; spill
    extra waits onto preceding NoOps on the same engine."""
    n = 0
    for fn in nc.m.functions:
        for bb in fn.blocks:
            new_insts = []
            for inst in bb.instructions:
                si = getattr(inst, "sync_info", None)
                if si is not None and si.on_wait and len(si.on_wait) > max_waits:
                    waits = list(si.on_wait)
                    extra, keep = waits[:-max_waits], waits[-max_waits:]
                    for w in extra:
                        nop = mybir.InstNoOp(
                            name=nc.get_next_instruction_name(),
                            sync_info=mybir.SyncInfo(on_wait=[w], on_update=[]),
                            bass_nofuse=True,
                            engine=inst.engine,
                        )
                        nc.register_instruction(nop)
                        new_insts.append(nop)
                        n += 1
                    si.on_wait = keep
                new_insts.append(inst)
            bb.instructions[:] = new_insts
    return n


def build_nc(reps=1):
    nc = bass.Bass()

    # xv pre-tiled on host so each t-chunk DMA balances to 3 AP dims
    xv = nc.declare_dram_parameter("xv", [DT_N, 3, TT_N, P, P], f8, isOutput=False)
    wv = nc.declare_dram_parameter("wv", [NG, D, 3, 384], f8, isOutput=False)
    woT = nc.declare_dram_parameter("woT", [H8 * DH, D], f16, isOutput=False)
    ve = nc.declare_dram_parameter("ve", [T, H8 * DH], f16, isOutput=False)
    cosd = nc.declare_dram_parameter("cosd", [T, 64], f16, isOutput=False)
    sind = nc.declare_dram_parameter("sind", [T, 64], f16, isOutput=False)
    amask = nc.declare_dram_parameter("amask", [P, P], f16, isOutput=False)
    bmask = nc.declare_dram_parameter("bmask", [P, P], f16, isOutput=False)
    out = nc.declare_dram_parameter("out", [T, D], f16, isOutput=True)

    xv_v = xv.rearrange("dt v tc p t -> p dt v tc t")
    wv_v = wv.rearrange("g (dt p) v r -> p g dt v r", p=P)
    wo_v = woT.rearrange("(jt p) i -> p jt i", p=P)
    ve_v = ve.rearrange("(tt p) r -> p tt r", p=P)
    cos_v = cosd.rearrange("(tt p) e -> p tt e", p=P)
    sin_v = sind.rearrange("(tt p) e -> p tt e", p=P)
    out_v = out.rearrange("(tt p) i -> p tt i", p=P)

    with tile.TileContext(nc) as tc:
        import contextlib

        with contextlib.ExitStack() as ctx:
            const = ctx.enter_context(tc.tile_pool(name="const", bufs=1))
            big = ctx.enter_context(tc.tile_pool(name="big", bufs=1))

            xv_sb = big.tile([P, DT_N, 3, T], f8)
            wv_sb = const.tile([P, NG, DT_N, 3, 384], f8)
            wo_sb = const.tile([P, JT_N, D], f16)
            ve_sb = const.tile([P, TT_N, H8 * DH], f16)
            cos_sb = const.tile([P, TT_N, 64], f16)
            sin_sb = const.tile([P, TT_N, 64], f16)
            am_sb = const.tile([P, P], f16)
            bm_sb = const.tile([P, P], f16)
            eps_sb = const.tile([P, 1], f32)

            vp = big.tile([P, KT_N, H8, DH + 1], f16)
            y16 = big.tile([P, TT_N, H8 * DH], f16)
            QT_sb = big.tile([P, NG, T], f16)
            KT_sb = big.tile([P, NG, T], f16)
            yT_sb = big.tile([P, JT_N, T], f16)

            # ---- initial loads, ordered by first use ----
            # x chunks 0-3 on the ACT queue (parallel with weights on SP)
            for c in range(4):
                nc.scalar.dma_start(out=xv_sb[:, :, :, ts(c, P)],
                                    in_=xv_v[:, :, :, c, :])
            nc.sync.dma_start(out=wv_sb[:, 0], in_=wv_v[:, 0])
            nc.sync.dma_start(out=am_sb[:], in_=amask[:])
            nc.sync.dma_start(out=bm_sb[:], in_=bmask[:])
            for c in range(4, 8):
                nc.sync.dma_start(out=xv_sb[:, :, :, ts(c, P)],
                                  in_=xv_v[:, :, :, c, :])
            nc.gpsimd.dma_start(out=ve_sb[:, 0:4, :], in_=ve_v[:, 0:4, :])
            nc.gpsimd.dma_start(out=cos_sb[:], in_=cos_v[:])
            nc.gpsimd.dma_start(out=sin_sb[:], in_=sin_v[:])
            nc.gpsimd.dma_start(out=ve_sb[:, 4:8, :], in_=ve_v[:, 4:8, :])
            nc.vector.memset(vp[:, :, :, DH], 1.0)
            nc.vector.memset(eps_sb[:], EPS)

            for _rep in range(reps):
                with tc.tile_pool(name="projps", bufs=2, space="PSUM") as psB, \
                     tc.tile_pool(name="stps", bufs=2, space="PSUM") as psST, \
                     tc.tile_pool(name="avps", bufs=2, space="PSUM") as psAV, \
                     tc.tile_pool(name="qk", bufs=2) as qkp, \
                     tc.tile_pool(name="sqp", bufs=2) as sqp, \
                     tc.tile_pool(name="etp", bufs=2) as etp, \
                     tc.tile_pool(name="stat", bufs=2) as statp:

                    QKgs = {}

                    def emit_proj(g):
                        gc = ts(g, P)
                        QKg = qkp.tile([P, TT_N, 2 * P], f16, tag="qkg")
                        QKgs[g] = QKg
                        sqg = sqp.tile([P, TT_N, 256], f16, tag="sqg")
                        for tt in range(TT_N):
                            pj = psB.tile([P, 384], f32, tag="pqkv")
                            first = True
                            for dp in range(DT_N // 2):
                                dsl = ds(2 * dp, 2)
                                for xvar, wvar in ((0, 0), (1, 2), (2, 1)):
                                    for c0, cn in ((0, 256), (256, 128)):
                                        nc.tensor.matmul(
                                            pj[:, ds(c0, cn)],
                                            xv_sb[:, dsl, xvar, ts(tt, P)],
                                            wv_sb[:, g, dsl, wvar, ds(c0, cn)],
                                            start=first,
                                            stop=(dp == 3 and xvar == 2
                                                  and c0 == 256),
                                            perf_mode=DR,
                                        )
                                        first = False
                            # v blend on DVE: vp = pj/64 + lam1*ve  (Pool
                            # cannot access PSUM per the walrus verifier)
                            nc.vector.scalar_tensor_tensor(
                                vp[:, tt, 2 * g : 2 * g + 2, 0:DH],
                                pj[:, 256:384].rearrange("p (h e) -> p h e", h=2),
                                1.0 / WSC,
                                ve_sb[:, tt, gc].rearrange("p (h e) -> p h e", h=2),
                                mybir.AluOpType.mult,
                                mybir.AluOpType.add,
                            )
                            # raw (64x-scaled) q,k copy: q half on ACT, k on DVE
                            nc.scalar.copy(out=QKg[:, tt, 0:P], in_=pj[:, 0:P])
                            nc.vector.tensor_copy(
                                out=QKg[:, tt, P : 2 * P], in_=pj[:, P : 2 * P])
                            # squares on DVE (fp16, 2x mode)
                            nc.vector.tensor_tensor(
                                sqg[:, tt, :], QKg[:, tt, :], QKg[:, tt, :],
                                mybir.AluOpType.mult,
                            )
                        # ---- batched RMS scale: scl = 1/(64*rms) ----
                        ms = statp.tile([P, TT_N, 4], f32, tag="ms")
                        nc.vector.reduce_sum(
                            ms[:],
                            sqg.rearrange("p tt (h e) -> p tt h e", h=4),
                            axis=mybir.AxisListType.X,
                        )
                        lnv = statp.tile([P, TT_N, 4], f32, tag="lnv")
                        nc.scalar.activation(
                            lnv[:], ms[:], mybir.ActivationFunctionType.Ln,
                            bias=eps_sb[:], scale=1.0 / DH,
                        )
                        scl = statp.tile([P, TT_N, 4], f32, tag="scl")
                        nc.scalar.activation(
                            scl[:], lnv[:], mybir.ActivationFunctionType.Exp,
                            scale=-0.5,
                        )
                        nc.vector.tensor_tensor(
                            QKg.rearrange("p tt (h e) -> p tt h e", h=4),
                            QKg.rearrange("p tt (h e) -> p tt h e", h=4),
                            scl[:, :, :, None].to_broadcast((P, TT_N, 4, DH)),
                            mybir.AluOpType.mult,
                        )
                        # ---- rotary: 4 DVE ops per (q|k) ----
                        cosv = cos_sb.rearrange("p tt (hh e) -> p tt hh e", hh=4)
                        sinv = sin_sb.rearrange("p tt (hh e) -> p tt hh e", hh=4)
                        for xi in range(2):
                            Xg = QKg[:, :, ds(xi * P, P)]
                            rot = Xg.rearrange(
                                "p tt (hh eh e) -> p tt hh eh e", hh=4, eh=2,
                            )[:, :, :, 0, :]  # [P, 8, 4, 16]
                            mcc = statp.tile([P, TT_N, 4, 16], f16, tag="mcc")
                            mss = statp.tile([P, TT_N, 4, 16], f16, tag="mss")
                            nc.vector.tensor_tensor(
                                mcc[:], rot, cosv, mybir.AluOpType.mult)
                            nc.vector.tensor_tensor(
                                mss[:], rot, sinv, mybir.AluOpType.mult)
                            nc.vector.tensor_tensor(
                                rot[:, :, 0::2, :], mcc[:, :, 0::2, :],
                                mss[:, :, 1::2, :], mybir.AluOpType.add)
                            nc.vector.tensor_tensor(
                                rot[:, :, 1::2, :], mcc[:, :, 1::2, :],
                                mss[:, :, 0::2, :], mybir.AluOpType.subtract)
                        # ---- transposes SBUF->SBUF on SP ----
                        for tt in range(TT_N):
                            nc.sync.dma_start_transpose(
                                QT_sb[:, g, ts(tt, P)], QKg[:, tt, 0:P])
                            nc.sync.dma_start_transpose(
                                KT_sb[:, g, ts(tt, P)], QKg[:, tt, P : 2 * P])

                    def emit_scores(g):
                        ET = etp.tile([P, 2, ETOFF[KT_N]], f16, tag="et")
                        for kt in range(KT_N):
                            qlo = kt * P
                            if qlo < 512:
                                chunks = [(qlo, 512), (512, T)]
                            else:
                                chunks = [(qlo, T)]
                            for qs, qe in chunks:
                                cols = qe - qs
                                pst = psST.tile([P, 2, 512], f32, tag="pst")
                                diag = qs == qlo
                                for hb in range(2):
                                    lo, hi = hb * 64, hb * 64 + 64
                                    nc.tensor.matmul(
                                        pst[:, hb, 0:cols],
                                        KT_sb[lo:hi, g, ts(kt, P)],
                                        QT_sb[lo:hi, g, ds(qs, cols)],
                                        start=True, stop=not diag,
                                    )
                                    if diag:
                                        nc.tensor.matmul(
                                            pst[:, hb, 0:P],
                                            am_sb[:], bm_sb[:],
                                            start=False, stop=True,
                                        )
                                nc.scalar.activation(
                                    ET[:, :, ds(ETOFF[kt] + qs - qlo, cols)],
                                    pst[:, :, 0:cols],
                                    mybir.ActivationFunctionType.Exp,
                                    scale=ATTN_SCALE,
                                )
                        return ET

                    def emit_av(g, ET):
                        gc = ts(g, P)
                        for q2 in range(TT_N // 2):
                            pav = psAV.tile([P, 260], f32, tag="av")
                            mms = []
                            for sub in range(2):
                                qt = 2 * q2 + sub
                                for hb in range(2):
                                    for kt in range(qt + 1):
                                        mms.append((sub, qt, hb, kt))
                            for i, (sub, qt, hb, kt) in enumerate(mms):
                                nc.tensor.matmul(
                                    pav[:, ds(sub * 130 + hb * 65, 65)],
                                    ET[:, hb, ds(ETOFF[kt] + (qt - kt) * P, P)],
                                    vp[:, kt, 2 * g + hb, :],
                                    start=(i == 0),
                                    stop=(i == len(mms) - 1),
                                )
                            pavv = pav.rearrange("p (s h c) -> p s h c", s=2, h=2)
                            r = statp.tile([P, 2, 2], f32, tag="r")
                            nc.vector.reciprocal(r[:], pavv[:, :, :, DH : DH + 1])
                            nc.vector.tensor_tensor(
                                y16[:, ds(2 * q2, 2), gc].rearrange(
                                    "p s (h e) -> p s h e", h=2),
                                pavv[:, :, :, 0:DH],
                                r[:, :, :, None].to_broadcast((P, 2, 2, DH)),
                                mybir.AluOpType.mult,
                            )
                            for sub in range(2):
                                tt = 2 * q2 + sub
                                nc.sync.dma_start_transpose(
                                    yT_sb[:, g, ts(tt, P)], y16[:, tt, gc])

                    # ---------- software-pipelined emission ----------
                    emit_proj(0)
                    nc.sync.dma_start(out=wv_sb[:, 1], in_=wv_v[:, 1])
                    ET0 = emit_scores(0)
                    emit_proj(1)
                    nc.sync.dma_start(out=wv_sb[:, 2], in_=wv_v[:, 2])
                    emit_av(0, ET0)
                    ET1 = emit_scores(1)
                    emit_proj(2)
                    nc.sync.dma_start(out=wv_sb[:, 3], in_=wv_v[:, 3])
                    nc.sync.dma_start(out=wo_sb[:], in_=wo_v[:])
                    emit_av(1, ET1)
                    ET2 = emit_scores(2)
                    emit_proj(3)
                    emit_av(2, ET2)
                    ET3 = emit_scores(3)
                    emit_av(3, ET3)

                # ================= output projection =====================
                with tc.tile_pool(name="outps", bufs=2, space="PSUM") as psF, \
                     tc.tile_pool(name="outstage", bufs=2) as osp:
                    for tt2 in range(TT_N // 2):
                        osb = osp.tile([P, 2, D], f16, tag="osb")
                        for sub in range(2):
                            tt = 2 * tt2 + sub
                            for ic in range(2):
                                po = psF.tile([P, 512], f32, tag="po")
                                for jt in range(JT_N):
                                    nc.tensor.matmul(
                                        po[:],
                                        yT_sb[:, jt, ts(tt, P)],
                                        wo_sb[:, jt, ds(ic * 512, 512)],
                                        start=(jt == 0), stop=(jt == JT_N - 1),
                                    )
                                nc.scalar.copy(
                                    out=osb[:, sub, ds(ic * 512, 512)], in_=po[:]
                                )
                        nc.gpsimd.dma_start(
                            out=out_v[:, ds(2 * tt2, 2), :], in_=osb[:]
                        )

    split_sync_waits(nc)
    return nc


def _f8(a):
    import ml_dtypes

    F8 = getattr(ml_dtypes, "float8_e4m3fn", None) or ml_dtypes.float8_e4m3
    return np.asarray(a, np.float32).astype(F8)


def _variants(m):
    """(hi, hi/16, lo*16) fp8 triplet stacked on a new axis 1."""
    hi = _f8(m)
    hi16 = _f8(np.asarray(m, np.float32) / 16.0)
    lo16 = _f8((np.asarray(m, np.float32) - hi.astype(np.float32)) * 16.0)
    return np.stack([hi, hi16, lo16], axis=1)


def make_core_inputs(x, qkvo_w, value_embeds, lambda_v):
    """Host-side prep: returns list of per-core input dicts."""
    x = np.asarray(x)
    qkvo_w = np.asarray(qkvo_w)
    value_embeds = np.asarray(value_embeds)
    lambda_v = np.asarray(lambda_v)

    freq = (1.0 / 1024.0) ** np.linspace(0.0, 1.0, DH // 4, dtype=np.float32)
    theta = np.arange(T, dtype=np.float32)[:, None] * freq[None, :]  # [T, 16]
    cos = np.cos(theta).astype(np.float32)
    sin = np.sin(theta).astype(np.float32)
    cos64 = np.concatenate([cos, cos, cos, cos], axis=1).astype(np.float16)
    sin64 = np.concatenate([sin, sin, sin, sin], axis=1).astype(np.float16)
    jj = np.arange(P)
    amask_np = (jj[None, :] >= jj[:, None]).astype(np.float16)   # [j, k]
    bmask_np = (-2000.0 * (jj[:, None] > jj[None, :])).astype(np.float16)  # [j, q]

    in_maps = []
    for c in range(N_CORES):
        b, hh = c // 2, c % 2
        R = slice(hh * H8 * DH, (hh + 1) * H8 * DH)
        wq = qkvo_w[0][R].T * WSC  # [D, 512]
        wk = qkvo_w[1][R].T * WSC
        wvv = (lambda_v[0] * qkvo_w[2][R]).T * WSC
        # [D, NG, 384]: per pair the 128 q cols, 128 k cols, 128 v cols
        wqkv = np.empty((D, NG, 384), dtype=np.float32)
        for g in range(NG):
            wqkv[:, g, 0:128] = wq[:, g * 128 : (g + 1) * 128]
            wqkv[:, g, 128:256] = wk[:, g * 128 : (g + 1) * 128]
            wqkv[:, g, 256:384] = wvv[:, g * 128 : (g + 1) * 128]
        xT = np.ascontiguousarray(x[b].T)  # [D, T] fp32
        # xv: [D,3,T] -> [dt, 3, tc, p, t] so t-chunk DMAs balance to 3 dims
        xv3 = _variants(xT).reshape(DT_N, P, 3, TT_N, P).transpose(0, 2, 3, 1, 4)
        # wv: [D,3,NG,384] -> g-major [NG, D, 3, 384]
        wv3 = _variants(wqkv).transpose(2, 0, 1, 3)
        in_maps.append({
            "xv": np.ascontiguousarray(xv3),           # [dt, 3, tc, p, t] fp8
            "wv": np.ascontiguousarray(wv3),           # [NG, D, 3, 384] fp8
            "woT": np.ascontiguousarray(qkvo_w[3][:, R].T).astype(np.float16),
            "ve": (lambda_v[1] * value_embeds[:T, R]).astype(np.float16),
            "cosd": cos64,
            "sind": sin64,
            "amask": amask_np,
            "bmask": bmask_np,
        })
    return in_maps


_NC_CACHE = {}


def _get_nc(reps=1):
    if reps not in _NC_CACHE:
        _NC_CACHE[reps] = build_nc(reps)
    return _NC_CACHE[reps]


def kernel(x, qkvo_w, value_embeds, lambda_v):
    from concourse.bass_utils import run_bass_kernel_spmd

    nc = _get_nc()
    in_maps = make_core_inputs(x, qkvo_w, value_embeds, lambda_v)
    res = run_bass_kernel_spmd(nc, in_maps, list(range(N_CORES))).results
    out = np.empty((B, T, D), dtype=np.float32)
    for b in range(B):
        out[b] = (res[2 * b]["out"].astype(np.float32)
                  + res[2 * b + 1]["out"].astype(np.float32))
    return out
